# revision 11
# baseline (speedup 1.0000x reference)
"""Trainium2 Bass kernel for GQA attention (B=1, S=2048, D=2048, H=32, KV=8, HD=64).

Tensor-parallel over heads across 8 NeuronCores: core i holds q-heads
[4i, 4i+4) and kv-head i; each core computes its partial o_proj output and the
host sums the 8 partials (Megatron all-reduce done host-side).

Pipelined emission: q-block-major stages; each attention unit's scores/exp run
one unit ahead of its PV matmuls, with projection and o_proj matmuls
interleaved as PE filler so the tensor engine never idles (keeps HAM warm).

Self-contained: only imports concourse (on sys.path in the container).
"""

import math
import os
import sys

import ml_dtypes
import numpy as np

if "/opt/trn_rl_repo" not in sys.path and not any(
    p.endswith("trn_rl_repo") for p in sys.path
):
    sys.path.insert(0, "/opt/trn_rl_repo")

import concourse.bass as bass
import concourse.mybir as mybir
import concourse.tile as tile
from concourse import bacc
from concourse.bass_utils import run_bass_kernel_spmd
from concourse.masks import make_identity

F32 = mybir.dt.float32
BF16 = mybir.dt.bfloat16

AF = mybir.ActivationFunctionType
ALU = mybir.AluOpType

S = 2048
D = 2048
H = 32
KV = 8
HD = 64
NCORES = 8
HQ = H // NCORES  # 4 q heads per core
NQB = 4  # q blocks of 512
QBW = 512
NSB = 4  # s blocks of 512 in projection
SBW = 512
DCH = D // 128  # 16 contraction chunks


def _build_nc():
    nc = bacc.Bacc("TRN2", target_bir_lowering=False, debug=False, num_devices=NCORES)

    xt_d = nc.declare_dram_parameter("xt", [D, S], BF16, isOutput=False)
    wqkv_d = nc.declare_dram_parameter("wqkv", [D, 384], BF16, isOutput=False)
    wo_d = nc.declare_dram_parameter("wo", [2, 128, D], BF16, isOutput=False)
    cos_d = nc.declare_dram_parameter("cos", [128, S], BF16, isOutput=False)
    sin_d = nc.declare_dram_parameter("sin", [128, S], BF16, isOutput=False)
    sel_d = nc.declare_dram_parameter("sel", [16, HQ * 128], BF16, isOutput=False)
    y_d = nc.declare_dram_parameter("y", [S, D], BF16, isOutput=True)

    with tile.TileContext(nc) as tc:
        with (
            tc.tile_pool(name="glob", bufs=1) as glob,
        ):
            ktdup = glob.tile([128, S], BF16, tag="ktdup")
            v_s = glob.tile([128, 16, 65], BF16, tag="v_s")
            outA = glob.tile([128, S], BF16, tag="outA")
            outB = glob.tile([128, S], BF16, tag="outB")
            ao = glob.tile([128, 2, S], BF16, tag="ao")
            sel_s = glob.tile([16, HQ * 128], BF16, tag="sel_s")
            ident = glob.tile([128, 128], F32, tag="ident")
            wo_s = glob.tile([128, 2, D], BF16, tag="wo_s")
            sums_q = glob.tile([16, NQB, QBW], F32, tag="sums_q")
            rcp_all = glob.tile([16, NQB, QBW], BF16, tag="rcp_all")
            rcp_f32 = glob.tile([16, QBW], F32, tag="rcp_f32")
            rcp_scr = glob.tile([16, QBW], F32, tag="rcp_scr")
            qs_all = [
                glob.tile([128, S], BF16, tag="qs", name=f"qs{i}", bufs=HQ)
                for i in range(HQ)
            ]

            nc.vector.memset(v_s[:, :, 64], 1.0)
            nc.vector.memset(sums_q[:], 1.0)

            with (
                tc.tile_pool(name="p1", bufs=1) as p1,
                tc.tile_pool(name="xp", bufs=2) as xp,
                tc.tile_pool(name="tmpp", bufs=4) as tmpp,
                tc.tile_pool(name="ptp", bufs=18) as ptp,
                tc.tile_pool(name="stgp", bufs=4) as stgp,
                tc.tile_pool(name="yp", bufs=6) as yp,
                tc.tile_pool(name="pssc", bufs=2, space="PSUM") as pssc,
                tc.tile_pool(name="psop", bufs=1, space="PSUM") as psop,
                tc.tile_pool(name="psml", bufs=3, space="PSUM") as psml,
            ):
                wq_s = p1.tile([128, DCH, 384], BF16, tag="wq_s")
                wqkv_r = wqkv_d.rearrange("(ko p) n -> p ko n", p=128)
                cos_s = p1.tile([128, S], BF16, tag="cos_s")
                sin_s = p1.tile([128, S], BF16, tag="sin_s")
                kvraw = p1.tile([128, S], F32, tag="kvraw")
                kswap = p1.tile([64, S], F32, tag="kswap")
                xt_r = xt_d.rearrange("(ko p) s -> p ko s", p=128)

                xblks = {}

                def issue_xblk(sb):
                    sbc = slice(sb * SBW, (sb + 1) * SBW)
                    xblk = xp.tile([128, DCH, SBW], BF16, tag="xblk", name=f"xblk{sb}")
                    for kq in range(4):
                        eng = nc.sync if kq % 2 == 0 else nc.gpsimd
                        eng.dma_start(
                            xblk[:, 4 * kq : 4 * kq + 4, :],
                            xt_r[:, 4 * kq : 4 * kq + 4, sbc],
                        )
                    xblks[sb] = xblk

                # ---------- projection for s-block sb, split into filler steps ----
                def proj_closures(sb):
                    sbc = slice(sb * SBW, (sb + 1) * SBW)
                    st = {}

                    def c_kv():
                        psKV = psml.tile([128, SBW], F32, tag="ps", name=f"psKV{sb}")
                        for kc in range(DCH):
                            nc.tensor.matmul(
                                psKV[:],
                                lhsT=wq_s[:, kc, 256:384],
                                rhs=xblks[sb][:, kc, :],
                                start=(kc == 0),
                                stop=(kc == DCH - 1),
                            )
                        nc.scalar.activation(kvraw[:, sbc], psKV[:], AF.Copy)

                    def c_a():
                        st["psA"] = psml.tile([128, SBW], F32, tag="ps", name=f"psA{sb}")
                        for kc in range(DCH):
                            nc.tensor.matmul(
                                st["psA"][:],
                                lhsT=wq_s[:, kc, 0:128],
                                rhs=xblks[sb][:, kc, :],
                                start=(kc == 0),
                                stop=(kc == DCH - 1),
                            )

                    def c_b():
                        st["psB"] = psml.tile([128, SBW], F32, tag="ps", name=f"psB{sb}")
                        for kc in range(DCH):
                            nc.tensor.matmul(
                                st["psB"][:],
                                lhsT=wq_s[:, kc, 128:256],
                                rhs=xblks[sb][:, kc, :],
                                start=(kc == 0),
                                stop=(kc == DCH - 1),
                            )

                    def c_ropeA():
                        psA, psB = st["psA"], st["psB"]
                        tmp = tmpp.tile([128, SBW], F32, tag="tmp", name=f"tmpA{sb}")
                        nc.vector.tensor_tensor(
                            outA[:, sbc], psA[:], cos_s[:, sbc], ALU.mult
                        )
                        nc.vector.tensor_tensor(tmp[:], psB[:], sin_s[:, sbc], ALU.mult)
                        nc.vector.tensor_tensor(
                            outA[:, sbc], outA[:, sbc], tmp[:], ALU.subtract
                        )

                    def c_ropeB():
                        psA, psB = st["psA"], st["psB"]
                        tmp2 = tmpp.tile([128, SBW], F32, tag="tmp", name=f"tmpB{sb}")
                        nc.vector.tensor_tensor(
                            outB[:, sbc], psB[:], cos_s[:, sbc], ALU.mult
                        )
                        nc.vector.tensor_tensor(
                            tmp2[:], psA[:], sin_s[:, sbc], ALU.mult
                        )
                        nc.vector.tensor_tensor(
                            outB[:, sbc], outB[:, sbc], tmp2[:], ALU.add
                        )

                    def c_krope():
                        nc.sync.dma_start(kswap[0:32, sbc], kvraw[32:64, sbc])
                        nc.sync.dma_start(kswap[32:64, sbc], kvraw[0:32, sbc])
                        nc.vector.tensor_tensor(
                            ktdup[0:64, sbc],
                            kvraw[0:64, sbc],
                            cos_s[0:64, sbc],
                            ALU.mult,
                        )
                        tmpk = tmpp.tile([64, SBW], F32, tag="tmpk", name=f"tmpk{sb}")
                        nc.vector.tensor_tensor(
                            tmpk[:], kswap[:, sbc], sin_s[0:64, sbc], ALU.mult
                        )
                        nc.vector.tensor_tensor(
                            ktdup[0:32, sbc],
                            ktdup[0:32, sbc],
                            tmpk[0:32, :],
                            ALU.subtract,
                        )
                        nc.vector.tensor_tensor(
                            ktdup[32:64, sbc],
                            ktdup[32:64, sbc],
                            tmpk[32:64, :],
                            ALU.add,
                        )
                        nc.sync.dma_start(ktdup[64:128, sbc], ktdup[0:64, sbc])

                    def c_vtrans():
                        for c in range(4 * sb, 4 * sb + 4):
                            ptr = psml.tile([128, SBW], F32, tag="ps", name=f"ptr{sb}_{c}")
                            nc.tensor.transpose(
                                ptr[:, 0:64],
                                kvraw[64:128, c * 128 : (c + 1) * 128],
                                ident[64:128, 64:128],
                            )
                            nc.vector.tensor_copy(v_s[:, c, 0:64], ptr[:, 0:64])

                    def c_qstage():
                        for h in range(HQ):
                            hc = slice(32 * h, 32 * h + 32)
                            qs = qs_all[h]
                            eng = nc.sync if h % 2 == 0 else nc.gpsimd
                            eng.dma_start(qs[0:32, sbc], outA[hc, sbc])
                            eng.dma_start(qs[32:64, sbc], outB[hc, sbc])
                            eng.dma_start(qs[64:96, sbc], outA[hc, sbc])
                            eng.dma_start(qs[96:128, sbc], outB[hc, sbc])

                    return [c_kv, c_a, c_b, c_ropeA, c_ropeB, c_krope, c_vtrans, c_qstage]

                # ---------------- attention unit: front (scores+exp+mask) --------
                def emit_front(h, qb):
                    qs = qs_all[h]
                    q0 = qb * QBW
                    nkc = 4 * (qb + 1)
                    pairs = []
                    for pair in range(nkc // 2):
                        cA, cB = 2 * pair, 2 * pair + 1
                        psc = pssc.tile([128, 1024], F32, tag="psc", name=f"psc{h}_{qb}_{pair}")
                        ptt = ptp.tile([128, 1024], BF16, tag="ptt", name=f"ptt{h}_{qb}_{pair}")
                        for c, half, r0 in ((cA, 0, 0), (cB, 1, 64)):
                            kc0 = c * 128
                            d = max(0, kc0 - q0)
                            nc.tensor.matmul(
                                psc[:, half * 512 + d : half * 512 + 512],
                                lhsT=ktdup[r0 : r0 + 64, kc0 : kc0 + 128],
                                rhs=qs[r0 : r0 + 64, q0 + d : q0 + QBW],
                                start=True,
                                stop=True,
                                tile_position=(r0, 0),
                            )
                        dA = max(0, cA * 128 - q0)
                        # single exp over [dA:1024]; the invalid middle region
                        # becomes garbage exp values that affine_select zeroes.
                        nc.scalar.activation(ptt[:, dA:1024], psc[:, dA:1024], AF.Exp)
                        for c, half in ((cA, 0), (cB, 1)):
                            kc0 = c * 128
                            if kc0 + 127 > q0:
                                ww = min(512, (kc0 - q0) + 128)
                                sl = slice(half * 512, half * 512 + ww)
                                nc.gpsimd.affine_select(
                                    out=ptt[:, sl],
                                    in_=ptt[:, sl],
                                    compare_op=ALU.is_ge,
                                    fill=0.0,
                                    base=q0 - kc0,
                                    channel_multiplier=-1,
                                    pattern=[[1, ww]],
                                )
                        pairs.append(ptt)
                    return pairs

                # ---------------- attention unit: back (PV + eviction) -----------
                def emit_back(h, qb, pairs):
                    nkc = 4 * (qb + 1)
                    pso = psop.tile([128, QBW], F32, tag="pso", name=f"pso{h}_{qb}")
                    for pair, ptt in enumerate(pairs):
                        for c, half in ((2 * pair, 0), (2 * pair + 1, 1)):
                            nc.tensor.matmul(
                                pso[0:65, :],
                                lhsT=v_s[:, c, :],
                                rhs=ptt[:, half * 512 : half * 512 + 512],
                                start=(c == 0),
                                stop=(c == nkc - 1),
                            )
                    ch = h // 2
                    rr = 64 * (h % 2)
                    stg = stgp.tile([128, QBW], BF16, tag="stg", name=f"stg{h}_{qb}")
                    nc.vector.tensor_copy(stg[0:64, :], pso[0:64, :])
                    sumr = stgp.tile([128, QBW], F32, tag="sumr", name=f"sumr{h}_{qb}")
                    nc.vector.tensor_copy(sumr[64:65, :], pso[64:65, :])
                    nc.sync.dma_start(
                        ao[rr : rr + 64, ch, qb * QBW : (qb + 1) * QBW], stg[0:64, :]
                    )
                    nc.gpsimd.dma_start(sums_q[h : h + 1, qb, :], sumr[64:65, :])

                def emit_rcp(qb):
                    nc.vector.reciprocal_approx_accurate(
                        rcp_f32[:], sums_q[:, qb, :], rcp_scr[:]
                    )
                    nc.vector.tensor_copy(rcp_all[:, qb, :], rcp_f32[:])

                def norm_closure(h, qb):
                    def c():
                        ch = h // 2
                        rr = 64 * (h % 2)
                        q0 = qb * QBW
                        pbc = psml.tile([128, QBW], F32, tag="ps", name=f"pbc{h}_{qb}")
                        nc.tensor.matmul(
                            pbc[:],
                            lhsT=sel_s[:, h * 128 : (h + 1) * 128],
                            rhs=rcp_all[:, qb, :],
                            start=True,
                            stop=True,
                        )
                        nc.vector.tensor_tensor(
                            ao[rr : rr + 64, ch, q0 : q0 + QBW],
                            ao[rr : rr + 64, ch, q0 : q0 + QBW],
                            pbc[rr : rr + 64, :],
                            ALU.mult,
                        )

                    return c

                def oproj_closure(qb, i, act_every=3):
                    def c():
                        st = qb * 4 + i // 4
                        ob = i % 4
                        psy = psml.tile([128, QBW], F32, tag="ps", name=f"psy{qb}_{i}")
                        for chh in range(2):
                            nc.tensor.matmul(
                                psy[:],
                                lhsT=ao[:, chh, st * 128 : (st + 1) * 128],
                                rhs=wo_s[:, chh, ob * 512 : (ob + 1) * 512],
                                start=(chh == 0),
                                stop=(chh == 1),
                            )
                        ysb = yp.tile([128, QBW], BF16, tag="ysb", name=f"ysb{qb}_{i}")
                        if i % act_every == 0:
                            nc.scalar.activation(ysb[:], psy[:], AF.Copy)
                        else:
                            nc.vector.tensor_copy(ysb[:], psy[:])
                        eng = nc.gpsimd if i % 2 == 0 else nc.sync
                        eng.dma_start(
                            y_d[st * 128 : (st + 1) * 128, ob * 512 : (ob + 1) * 512],
                            ysb[:],
                        )

                    return c

                # ======================= prologue ================================
                for kc in range(DCH):
                    eng = nc.sync if kc % 2 == 0 else nc.gpsimd
                    eng.dma_start(wq_s[:, kc, :], wqkv_r[:, kc, :])
                issue_xblk(0)
                nc.sync.dma_start(cos_s[:], cos_d[:])
                nc.sync.dma_start(sin_s[:], sin_d[:])
                nc.gpsimd.dma_start(sel_s[:], sel_d[:])
                for chh in range(2):
                    nc.gpsimd.dma_start(wo_s[:, chh, :], wo_d[chh])
                make_identity(nc, ident[:])
                for c in proj_closures(0):
                    c()
                issue_xblk(1)

                # ======================= pipelined stages ========================
                pending = [None]

                def flush_pending():
                    if pending[0] is not None:
                        emit_back(*pending[0])
                        pending[0] = None

                for qb in range(NQB):
                    fillers = []
                    if qb == 0:
                        fillers.append(lambda: issue_xblk(2))
                        fillers.extend(proj_closures(1))
                    elif qb == 1:
                        fillers.append(lambda: issue_xblk(3))
                        fillers.extend(proj_closures(2))
                        fillers.extend(norm_closure(h2, 0) for h2 in range(HQ))
                    elif qb == 2:
                        fillers.extend(proj_closures(3))
                        fillers.extend(norm_closure(h2, 1) for h2 in range(HQ))
                        fillers.extend(oproj_closure(0, i) for i in range(16))
                    else:
                        fillers.extend(norm_closure(h2, 2) for h2 in range(HQ))
                        fillers.extend(oproj_closure(1, i) for i in range(16))

                    for h in range(HQ):
                        pairs = emit_front(h, qb)
                        flush_pending()
                        pending[0] = (h, qb, pairs)
                        if h == 0 and qb > 0:
                            emit_rcp(qb - 1)
                        npop = math.ceil(len(fillers) / (HQ - h))
                        for c in fillers[:npop]:
                            c()
                        fillers = fillers[npop:]
                    for c in fillers:
                        c()

                # ======================= epilogue ================================
                flush_pending()
                for i in range(4):
                    oproj_closure(2, i, act_every=2)()
                emit_rcp(3)
                for i in range(4, 16):
                    oproj_closure(2, i, act_every=2)()
                    if i % 4 == 3:
                        norm_closure(i // 4, 3)()
                norm_closure(0, 3)()
                for i in range(16):
                    oproj_closure(3, i, act_every=2)()
    nc.compile()
    return nc


def _prep_inputs(x, Wq, Wk, Wv, Wo, inv_freq):
    """Host-side sharding + layout prep. Returns in_maps for the 8 cores."""
    x = np.ascontiguousarray(np.asarray(x, dtype=np.float32).reshape(S, D))
    xt = np.ascontiguousarray(x.T)  # [D, S]

    pos = np.arange(S, dtype=np.float64)
    inv = np.asarray(inv_freq, dtype=np.float64)  # [32]
    freqs = pos[None, :] * inv[:, None]  # [32, S]
    cos32 = np.cos(freqs).astype(np.float32)
    sin32 = np.sin(freqs).astype(np.float32)
    cos_tab = np.tile(cos32, (4, 1))  # [128, S]
    sin_tab = np.tile(sin32, (4, 1))
    sel = np.zeros((16, HQ * 128), dtype=np.float32)
    for h in range(HQ):
        rr = 64 * (h % 2)
        sel[h, h * 128 + rr : h * 128 + rr + 64] = 1.0

    in_maps = []
    for i in range(NCORES):
        wq_l = Wq[256 * i : 256 * (i + 1)].astype(np.float32) * 0.125  # [256, D]
        wk_l = Wk[64 * i : 64 * (i + 1)].astype(np.float32)  # [64, D]
        wv_l = Wv[64 * i : 64 * (i + 1)].astype(np.float32)  # [64, D]
        # A-tile: first-half dims of the 4 heads; B-tile: second halves
        wA = np.concatenate(
            [wq_l[64 * h : 64 * h + 32] for h in range(HQ)], axis=0
        )  # [128, D]
        wB = np.concatenate(
            [wq_l[64 * h + 32 : 64 * h + 64] for h in range(HQ)], axis=0
        )
        wkv = np.concatenate([wk_l, wv_l], axis=0)  # [128, D]
        wqkv = np.ascontiguousarray(
            np.concatenate([wA, wB, wkv], axis=0).T
        )  # [D, 384]
        wo_l = Wo[:, 256 * i : 256 * (i + 1)].astype(np.float32)  # [D, 256]
        wo_t = np.ascontiguousarray(wo_l.T.reshape(2, 128, D))  # [2, 128, D]
        in_maps.append(
            {
                "xt": xt.astype(ml_dtypes.bfloat16),
                "wqkv": wqkv.astype(ml_dtypes.bfloat16),
                "wo": wo_t.astype(ml_dtypes.bfloat16),
                "cos": cos_tab.astype(ml_dtypes.bfloat16),
                "sin": sin_tab.astype(ml_dtypes.bfloat16),
                "sel": sel.astype(ml_dtypes.bfloat16),
            }
        )
    return in_maps


_NC_CACHE = None


def kernel(x, Wq, Wk, Wv, Wo, inv_freq):
    global _NC_CACHE
    if _NC_CACHE is None:
        _NC_CACHE = _build_nc()
    nc = _NC_CACHE
    in_maps = _prep_inputs(x, Wq, Wk, Wv, Wo, inv_freq)
    trace = bool(int(os.environ.get("BASS_KERNEL_TRACE", "0")))
    res = None
    last_exc = None
    for attempt in range(3):
        try:
            res = run_bass_kernel_spmd(nc, in_maps, list(range(NCORES)), trace=trace)
            break
        except Exception as e:  # transient device faults (rare) — retry
            last_exc = e
            msg = str(e)
            if "UNRECOVERABLE" in msg or "UNAVAILABLE" in msg or "Timeout" in msg:
                continue
            raise
    if res is None:
        raise last_exc
    if trace:
        kernel.last_results = res
    y = np.zeros((S, D), dtype=np.float32)
    for i in range(NCORES):
        y += res.results[i]["y"].astype(np.float32)
    return y.reshape(1, S, D)


# revision 14
# speedup vs baseline: 1.0937x; 1.0937x over previous
"""Trainium2 Bass kernel for GQA attention (B=1, S=2048, D=2048, H=32, KV=8, HD=64).

Tensor-parallel over heads across 8 NeuronCores: core i holds q-heads
[4i, 4i+4) and kv-head i; each core computes its partial o_proj output and the
host sums the 8 partials (Megatron all-reduce done host-side).

Pipelined emission: q-block-major stages; each attention unit's scores/exp run
one unit ahead of its PV matmuls, with projection and o_proj matmuls
interleaved as PE filler so the tensor engine never idles (keeps HAM warm).

Self-contained: only imports concourse (on sys.path in the container).
"""

import math
import os
import sys

import ml_dtypes
import numpy as np

if "/opt/trn_rl_repo" not in sys.path and not any(
    p.endswith("trn_rl_repo") for p in sys.path
):
    sys.path.insert(0, "/opt/trn_rl_repo")

import concourse.bass as bass
import concourse.mybir as mybir
import concourse.tile as tile
from concourse import bacc
from concourse.bass_utils import run_bass_kernel_spmd
from concourse.masks import make_identity

F32 = mybir.dt.float32
BF16 = mybir.dt.bfloat16

AF = mybir.ActivationFunctionType
ALU = mybir.AluOpType

S = 2048
D = 2048
H = 32
KV = 8
HD = 64
NCORES = 8
HQ = H // NCORES  # 4 q heads per core
NQB = 4  # q blocks of 512
QBW = 512
NSB = 4  # s blocks of 512 in projection
SBW = 512
DCH = D // 128  # 16 contraction chunks


def _build_nc():
    nc = bacc.Bacc("TRN2", target_bir_lowering=False, debug=False, num_devices=NCORES)

    xt_d = nc.declare_dram_parameter("xt", [D, S], BF16, isOutput=False)
    wqkv_d = nc.declare_dram_parameter("wqkv", [D, 384], BF16, isOutput=False)
    wo_d = nc.declare_dram_parameter("wo", [2, 128, D], BF16, isOutput=False)
    cos_d = nc.declare_dram_parameter("cos", [128, S], BF16, isOutput=False)
    sin_d = nc.declare_dram_parameter("sin", [128, S], BF16, isOutput=False)
    sel_d = nc.declare_dram_parameter("sel", [16, HQ * 128], BF16, isOutput=False)
    y_d = nc.declare_dram_parameter("y", [S, D], BF16, isOutput=True)

    with tile.TileContext(nc) as tc:
        with (
            tc.tile_pool(name="glob", bufs=1) as glob,
        ):
            ktdup = glob.tile([128, S], BF16, tag="ktdup")
            v_s = glob.tile([128, 16, 65], BF16, tag="v_s")
            outA = glob.tile([128, S], BF16, tag="outA")
            outB = glob.tile([128, S], BF16, tag="outB")
            ao = glob.tile([128, 2, S], BF16, tag="ao")
            sel_s = glob.tile([16, HQ * 128], BF16, tag="sel_s")
            ident = glob.tile([128, 128], F32, tag="ident")
            wo_s = glob.tile([128, 2, D], BF16, tag="wo_s")
            sums_q = glob.tile([16, NQB, QBW], F32, tag="sums_q")
            rcp_all = glob.tile([16, NQB, QBW], BF16, tag="rcp_all")
            rcp_f32 = glob.tile([16, QBW], F32, tag="rcp_f32")
            rcp_scr = glob.tile([16, QBW], F32, tag="rcp_scr")
            qs_all = [
                glob.tile([128, S], BF16, tag="qs", name=f"qs{i}", bufs=HQ)
                for i in range(HQ)
            ]

            nc.vector.memset(v_s[:, :, 64], 1.0)
            nc.vector.memset(sums_q[:], 1.0)

            with (
                tc.tile_pool(name="p1", bufs=1) as p1,
                tc.tile_pool(name="xp", bufs=3) as xp,
                tc.tile_pool(name="tmpp", bufs=4) as tmpp,
                tc.tile_pool(name="ptp", bufs=16) as ptp,
                tc.tile_pool(name="stgp", bufs=2) as stgp,
                tc.tile_pool(name="yp", bufs=4) as yp,
                tc.tile_pool(name="pssc", bufs=2, space="PSUM") as pssc,
                tc.tile_pool(name="psop", bufs=1, space="PSUM") as psop,
                tc.tile_pool(name="psml", bufs=3, space="PSUM") as psml,
            ):
                wq_s = p1.tile([128, DCH, 384], BF16, tag="wq_s")
                wqkv_r = wqkv_d.rearrange("(ko p) n -> p ko n", p=128)
                cos_s = p1.tile([128, S], BF16, tag="cos_s")
                sin_s = p1.tile([128, S], BF16, tag="sin_s")
                kvraw = p1.tile([128, S], F32, tag="kvraw")
                kswap = p1.tile([64, S], F32, tag="kswap")
                xt_r = xt_d.rearrange("(ko p) s -> p ko s", p=128)

                xblks = {}

                def issue_xblk(sb):
                    sbc = slice(sb * SBW, (sb + 1) * SBW)
                    xblk = xp.tile([128, DCH, SBW], BF16, tag="xblk", name=f"xblk{sb}")
                    for kq in range(4):
                        eng = nc.sync if kq % 2 == 0 else nc.gpsimd
                        eng.dma_start(
                            xblk[:, 4 * kq : 4 * kq + 4, :],
                            xt_r[:, 4 * kq : 4 * kq + 4, sbc],
                        )
                    xblks[sb] = xblk

                # ---------- projection for s-block sb, split into filler steps ----
                def proj_closures(sb):
                    sbc = slice(sb * SBW, (sb + 1) * SBW)
                    st = {}

                    def c_kv():
                        psKV = psml.tile([128, SBW], F32, tag="ps", name=f"psKV{sb}")
                        for kc in range(DCH):
                            nc.tensor.matmul(
                                psKV[:],
                                lhsT=wq_s[:, kc, 256:384],
                                rhs=xblks[sb][:, kc, :],
                                start=(kc == 0),
                                stop=(kc == DCH - 1),
                            )
                        nc.scalar.activation(kvraw[:, sbc], psKV[:], AF.Copy)

                    def c_a():
                        st["psA"] = psml.tile([128, SBW], F32, tag="ps", name=f"psA{sb}")
                        for kc in range(DCH):
                            nc.tensor.matmul(
                                st["psA"][:],
                                lhsT=wq_s[:, kc, 0:128],
                                rhs=xblks[sb][:, kc, :],
                                start=(kc == 0),
                                stop=(kc == DCH - 1),
                            )

                    def c_b():
                        st["psB"] = psml.tile([128, SBW], F32, tag="ps", name=f"psB{sb}")
                        for kc in range(DCH):
                            nc.tensor.matmul(
                                st["psB"][:],
                                lhsT=wq_s[:, kc, 128:256],
                                rhs=xblks[sb][:, kc, :],
                                start=(kc == 0),
                                stop=(kc == DCH - 1),
                            )

                    def c_ropeA():
                        psA, psB = st["psA"], st["psB"]
                        tmp = tmpp.tile([128, SBW], F32, tag="tmp", name=f"tmpA{sb}")
                        nc.vector.tensor_tensor(
                            outA[:, sbc], psA[:], cos_s[:, sbc], ALU.mult
                        )
                        nc.vector.tensor_tensor(tmp[:], psB[:], sin_s[:, sbc], ALU.mult)
                        nc.vector.tensor_tensor(
                            outA[:, sbc], outA[:, sbc], tmp[:], ALU.subtract
                        )

                    def c_ropeB():
                        psA, psB = st["psA"], st["psB"]
                        tmp2 = tmpp.tile([128, SBW], F32, tag="tmp", name=f"tmpB{sb}")
                        nc.vector.tensor_tensor(
                            outB[:, sbc], psB[:], cos_s[:, sbc], ALU.mult
                        )
                        nc.vector.tensor_tensor(
                            tmp2[:], psA[:], sin_s[:, sbc], ALU.mult
                        )
                        nc.vector.tensor_tensor(
                            outB[:, sbc], outB[:, sbc], tmp2[:], ALU.add
                        )

                    def c_krope():
                        nc.sync.dma_start(kswap[0:32, sbc], kvraw[32:64, sbc])
                        nc.sync.dma_start(kswap[32:64, sbc], kvraw[0:32, sbc])
                        nc.vector.tensor_tensor(
                            ktdup[0:64, sbc],
                            kvraw[0:64, sbc],
                            cos_s[0:64, sbc],
                            ALU.mult,
                        )
                        tmpk = tmpp.tile([64, SBW], F32, tag="tmpk", name=f"tmpk{sb}")
                        nc.vector.tensor_tensor(
                            tmpk[:], kswap[:, sbc], sin_s[0:64, sbc], ALU.mult
                        )
                        nc.vector.tensor_tensor(
                            ktdup[0:32, sbc],
                            ktdup[0:32, sbc],
                            tmpk[0:32, :],
                            ALU.subtract,
                        )
                        nc.vector.tensor_tensor(
                            ktdup[32:64, sbc],
                            ktdup[32:64, sbc],
                            tmpk[32:64, :],
                            ALU.add,
                        )
                        nc.sync.dma_start(ktdup[64:128, sbc], ktdup[0:64, sbc])

                    def c_vtrans():
                        for c in range(4 * sb, 4 * sb + 4):
                            ptr = psml.tile([128, SBW], F32, tag="ps", name=f"ptr{sb}_{c}")
                            nc.tensor.transpose(
                                ptr[:, 0:64],
                                kvraw[64:128, c * 128 : (c + 1) * 128],
                                ident[64:128, 64:128],
                            )
                            nc.vector.tensor_copy(v_s[:, c, 0:64], ptr[:, 0:64])

                    def c_qstage():
                        for h in range(HQ):
                            hc = slice(32 * h, 32 * h + 32)
                            qs = qs_all[h]
                            eng = nc.sync if h % 2 == 0 else nc.gpsimd
                            eng.dma_start(qs[0:32, sbc], outA[hc, sbc])
                            eng.dma_start(qs[32:64, sbc], outB[hc, sbc])
                            eng.dma_start(qs[64:96, sbc], outA[hc, sbc])
                            eng.dma_start(qs[96:128, sbc], outB[hc, sbc])

                    return [c_kv, c_a, c_b, c_ropeA, c_ropeB, c_krope, c_vtrans, c_qstage]

                # ---------------- attention unit: front (scores+exp+mask) --------
                def emit_front(h, qb):
                    qs = qs_all[h]
                    q0 = qb * QBW
                    nkc = 4 * (qb + 1)
                    pairs = []
                    for pair in range(nkc // 2):
                        cA, cB = 2 * pair, 2 * pair + 1
                        psc = pssc.tile([128, 1024], F32, tag="psc", name=f"psc{h}_{qb}_{pair}")
                        ptt = ptp.tile([128, 1024], BF16, tag="ptt", name=f"ptt{h}_{qb}_{pair}")
                        for c, half, r0 in ((cA, 0, 0), (cB, 1, 64)):
                            kc0 = c * 128
                            d = max(0, kc0 - q0)
                            nc.tensor.matmul(
                                psc[:, half * 512 + d : half * 512 + 512],
                                lhsT=ktdup[r0 : r0 + 64, kc0 : kc0 + 128],
                                rhs=qs[r0 : r0 + 64, q0 + d : q0 + QBW],
                                start=True,
                                stop=True,
                                tile_position=(r0, 0),
                            )
                        dA = max(0, cA * 128 - q0)
                        # single exp over [dA:1024]; the invalid middle region
                        # becomes garbage exp values that affine_select zeroes.
                        nc.scalar.activation(ptt[:, dA:1024], psc[:, dA:1024], AF.Exp)
                        for c, half in ((cA, 0), (cB, 1)):
                            kc0 = c * 128
                            if kc0 + 127 > q0:
                                ww = min(512, (kc0 - q0) + 128)
                                sl = slice(half * 512, half * 512 + ww)
                                nc.gpsimd.affine_select(
                                    out=ptt[:, sl],
                                    in_=ptt[:, sl],
                                    compare_op=ALU.is_ge,
                                    fill=0.0,
                                    base=q0 - kc0,
                                    channel_multiplier=-1,
                                    pattern=[[1, ww]],
                                )
                        pairs.append(ptt)
                    return pairs

                # ---------------- attention unit: back (PV + eviction) -----------
                def emit_back(h, qb, pairs):
                    nkc = 4 * (qb + 1)
                    pso = psop.tile([128, QBW], F32, tag="pso", name=f"pso{h}_{qb}")
                    for pair, ptt in enumerate(pairs):
                        for c, half in ((2 * pair, 0), (2 * pair + 1, 1)):
                            nc.tensor.matmul(
                                pso[0:65, :],
                                lhsT=v_s[:, c, :],
                                rhs=ptt[:, half * 512 : half * 512 + 512],
                                start=(c == 0),
                                stop=(c == nkc - 1),
                            )
                    ch = h // 2
                    rr = 64 * (h % 2)
                    stg = stgp.tile([128, QBW], BF16, tag="stg", name=f"stg{h}_{qb}")
                    nc.vector.tensor_copy(stg[0:64, :], pso[0:64, :])
                    sumr = stgp.tile([128, QBW], F32, tag="sumr", name=f"sumr{h}_{qb}")
                    nc.vector.tensor_copy(sumr[64:65, :], pso[64:65, :])
                    nc.sync.dma_start(
                        ao[rr : rr + 64, ch, qb * QBW : (qb + 1) * QBW], stg[0:64, :]
                    )
                    nc.gpsimd.dma_start(sums_q[h : h + 1, qb, :], sumr[64:65, :])

                def emit_rcp(qb):
                    nc.vector.reciprocal_approx_accurate(
                        rcp_f32[:], sums_q[:, qb, :], rcp_scr[:]
                    )
                    nc.vector.tensor_copy(rcp_all[:, qb, :], rcp_f32[:])

                def norm_closure(h, qb):
                    def c():
                        ch = h // 2
                        rr = 64 * (h % 2)
                        q0 = qb * QBW
                        pbc = psml.tile([128, QBW], F32, tag="ps", name=f"pbc{h}_{qb}")
                        nc.tensor.matmul(
                            pbc[:],
                            lhsT=sel_s[:, h * 128 : (h + 1) * 128],
                            rhs=rcp_all[:, qb, :],
                            start=True,
                            stop=True,
                        )
                        nc.vector.tensor_tensor(
                            ao[rr : rr + 64, ch, q0 : q0 + QBW],
                            ao[rr : rr + 64, ch, q0 : q0 + QBW],
                            pbc[rr : rr + 64, :],
                            ALU.mult,
                        )

                    return c

                def oproj_closure(qb, i, act_every=3):
                    def c():
                        st = qb * 4 + i // 4
                        ob = i % 4
                        psy = psml.tile([128, QBW], F32, tag="ps", name=f"psy{qb}_{i}")
                        for chh in range(2):
                            nc.tensor.matmul(
                                psy[:],
                                lhsT=ao[:, chh, st * 128 : (st + 1) * 128],
                                rhs=wo_s[:, chh, ob * 512 : (ob + 1) * 512],
                                start=(chh == 0),
                                stop=(chh == 1),
                            )
                        ysb = yp.tile([128, QBW], BF16, tag="ysb", name=f"ysb{qb}_{i}")
                        if i % act_every == 0:
                            nc.scalar.activation(ysb[:], psy[:], AF.Copy)
                        else:
                            nc.vector.tensor_copy(ysb[:], psy[:])
                        eng = nc.gpsimd if i % 2 == 0 else nc.sync
                        eng.dma_start(
                            y_d[st * 128 : (st + 1) * 128, ob * 512 : (ob + 1) * 512],
                            ysb[:],
                        )

                    return c

                # ======================= prologue ================================
                for kc in range(DCH):
                    eng = nc.sync if kc % 2 == 0 else nc.gpsimd
                    eng.dma_start(wq_s[:, kc, :], wqkv_r[:, kc, :])
                issue_xblk(0)
                nc.sync.dma_start(cos_s[:], cos_d[:])
                nc.sync.dma_start(sin_s[:], sin_d[:])
                nc.gpsimd.dma_start(sel_s[:], sel_d[:])
                for chh in range(2):
                    nc.gpsimd.dma_start(wo_s[:, chh, :], wo_d[chh])
                make_identity(nc, ident[:])
                issue_xblk(1)
                for c in proj_closures(0):
                    c()
                issue_xblk(2)

                # ======================= pipelined stages ========================
                pending = [None]

                def flush_pending():
                    if pending[0] is not None:
                        emit_back(*pending[0])
                        pending[0] = None

                for qb in range(NQB):
                    fillers = []
                    if qb == 0:
                        fillers.append(lambda: issue_xblk(3))
                        fillers.extend(proj_closures(1))
                    elif qb == 1:
                        fillers.extend(proj_closures(2))
                        fillers.extend(norm_closure(h2, 0) for h2 in range(HQ))
                    elif qb == 2:
                        fillers.extend(proj_closures(3))
                        fillers.extend(norm_closure(h2, 1) for h2 in range(HQ))
                        fillers.extend(oproj_closure(0, i) for i in range(16))
                    else:
                        fillers.extend(norm_closure(h2, 2) for h2 in range(HQ))
                        fillers.extend(oproj_closure(1, i) for i in range(16))
                        fillers.extend(oproj_closure(2, i) for i in range(8))

                    for h in range(HQ):
                        pairs = emit_front(h, qb)
                        flush_pending()
                        pending[0] = (h, qb, pairs)
                        if h == 0 and qb > 0:
                            emit_rcp(qb - 1)
                        npop = math.ceil(len(fillers) / (HQ - h))
                        for c in fillers[:npop]:
                            c()
                        fillers = fillers[npop:]
                    for c in fillers:
                        c()

                # ======================= epilogue ================================
                flush_pending()
                emit_rcp(3)
                for i in range(8, 16):
                    oproj_closure(2, i, act_every=2)()
                    if i % 2 == 1:
                        norm_closure((i - 9) // 2, 3)()
                for i in range(16):
                    oproj_closure(3, i, act_every=2)()
    nc.compile()
    return nc


def _prep_inputs(x, Wq, Wk, Wv, Wo, inv_freq):
    """Host-side sharding + layout prep. Returns in_maps for the 8 cores."""
    x = np.ascontiguousarray(np.asarray(x, dtype=np.float32).reshape(S, D))
    xt = np.ascontiguousarray(x.T)  # [D, S]

    pos = np.arange(S, dtype=np.float64)
    inv = np.asarray(inv_freq, dtype=np.float64)  # [32]
    freqs = pos[None, :] * inv[:, None]  # [32, S]
    cos32 = np.cos(freqs).astype(np.float32)
    sin32 = np.sin(freqs).astype(np.float32)
    cos_tab = np.tile(cos32, (4, 1))  # [128, S]
    sin_tab = np.tile(sin32, (4, 1))
    sel = np.zeros((16, HQ * 128), dtype=np.float32)
    for h in range(HQ):
        rr = 64 * (h % 2)
        sel[h, h * 128 + rr : h * 128 + rr + 64] = 1.0

    in_maps = []
    for i in range(NCORES):
        wq_l = Wq[256 * i : 256 * (i + 1)].astype(np.float32) * 0.125  # [256, D]
        wk_l = Wk[64 * i : 64 * (i + 1)].astype(np.float32)  # [64, D]
        wv_l = Wv[64 * i : 64 * (i + 1)].astype(np.float32)  # [64, D]
        # A-tile: first-half dims of the 4 heads; B-tile: second halves
        wA = np.concatenate(
            [wq_l[64 * h : 64 * h + 32] for h in range(HQ)], axis=0
        )  # [128, D]
        wB = np.concatenate(
            [wq_l[64 * h + 32 : 64 * h + 64] for h in range(HQ)], axis=0
        )
        wkv = np.concatenate([wk_l, wv_l], axis=0)  # [128, D]
        wqkv = np.ascontiguousarray(
            np.concatenate([wA, wB, wkv], axis=0).T
        )  # [D, 384]
        wo_l = Wo[:, 256 * i : 256 * (i + 1)].astype(np.float32)  # [D, 256]
        wo_t = np.ascontiguousarray(wo_l.T.reshape(2, 128, D))  # [2, 128, D]
        in_maps.append(
            {
                "xt": xt.astype(ml_dtypes.bfloat16),
                "wqkv": wqkv.astype(ml_dtypes.bfloat16),
                "wo": wo_t.astype(ml_dtypes.bfloat16),
                "cos": cos_tab.astype(ml_dtypes.bfloat16),
                "sin": sin_tab.astype(ml_dtypes.bfloat16),
                "sel": sel.astype(ml_dtypes.bfloat16),
            }
        )
    return in_maps


_NC_CACHE = None


def kernel(x, Wq, Wk, Wv, Wo, inv_freq):
    global _NC_CACHE
    if _NC_CACHE is None:
        _NC_CACHE = _build_nc()
    nc = _NC_CACHE
    in_maps = _prep_inputs(x, Wq, Wk, Wv, Wo, inv_freq)
    trace = bool(int(os.environ.get("BASS_KERNEL_TRACE", "0")))
    res = None
    last_exc = None
    for attempt in range(3):
        try:
            res = run_bass_kernel_spmd(nc, in_maps, list(range(NCORES)), trace=trace)
            break
        except Exception as e:  # transient device faults (rare) — retry
            last_exc = e
            msg = str(e)
            if "UNRECOVERABLE" in msg or "UNAVAILABLE" in msg or "Timeout" in msg:
                continue
            raise
    if res is None:
        raise last_exc
    if trace:
        kernel.last_results = res
    y = np.zeros((S, D), dtype=np.float32)
    for i in range(NCORES):
        y += res.results[i]["y"].astype(np.float32)
    return y.reshape(1, S, D)


# revision 15
# speedup vs baseline: 1.1105x; 1.0153x over previous
"""Trainium2 Bass kernel for GQA attention (B=1, S=2048, D=2048, H=32, KV=8, HD=64).

Tensor-parallel over heads across 8 NeuronCores: core i holds q-heads
[4i, 4i+4) and kv-head i; each core computes its partial o_proj output and the
host sums the 8 partials (Megatron all-reduce done host-side).

Pipelined emission: q-block-major stages; each attention unit's scores/exp run
one unit ahead of its PV matmuls, with projection and o_proj matmuls
interleaved as PE filler so the tensor engine never idles (keeps HAM warm).

Self-contained: only imports concourse (on sys.path in the container).
"""

import math
import os
import sys

import ml_dtypes
import numpy as np

if "/opt/trn_rl_repo" not in sys.path and not any(
    p.endswith("trn_rl_repo") for p in sys.path
):
    sys.path.insert(0, "/opt/trn_rl_repo")

import concourse.bass as bass
import concourse.mybir as mybir
import concourse.tile as tile
from concourse import bacc
from concourse.bass_utils import run_bass_kernel_spmd
from concourse.masks import make_identity

F32 = mybir.dt.float32
BF16 = mybir.dt.bfloat16

AF = mybir.ActivationFunctionType
ALU = mybir.AluOpType

S = 2048
D = 2048
H = 32
KV = 8
HD = 64
NCORES = 8
HQ = H // NCORES  # 4 q heads per core
NQB = 4  # q blocks of 512
QBW = 512
NSB = 4  # s blocks of 512 in projection
SBW = 512
DCH = D // 128  # 16 contraction chunks


def _build_nc():
    nc = bacc.Bacc("TRN2", target_bir_lowering=False, debug=False, num_devices=NCORES)

    xt_d = nc.declare_dram_parameter("xt", [D, S], BF16, isOutput=False)
    wqkv_d = nc.declare_dram_parameter("wqkv", [D, 384], BF16, isOutput=False)
    wo_d = nc.declare_dram_parameter("wo", [2, 128, D], BF16, isOutput=False)
    cos_d = nc.declare_dram_parameter("cos", [128, S], BF16, isOutput=False)
    sin_d = nc.declare_dram_parameter("sin", [128, S], BF16, isOutput=False)
    sel_d = nc.declare_dram_parameter("sel", [16, HQ * 128], BF16, isOutput=False)
    y_d = nc.declare_dram_parameter("y", [S, D], BF16, isOutput=True)

    with tile.TileContext(nc) as tc:
        with (
            tc.tile_pool(name="glob", bufs=1) as glob,
        ):
            ktdup = glob.tile([128, S], BF16, tag="ktdup")
            v_s = glob.tile([128, 16, 65], BF16, tag="v_s")
            outA = glob.tile([128, S], BF16, tag="outA")
            outB = glob.tile([128, S], BF16, tag="outB")
            ao = glob.tile([128, 2, S], BF16, tag="ao")
            sel_s = glob.tile([16, HQ * 128], BF16, tag="sel_s")
            ident = glob.tile([128, 128], F32, tag="ident")
            wo_s = glob.tile([128, 2, D], BF16, tag="wo_s")
            sums_q = glob.tile([16, NQB, QBW], F32, tag="sums_q")
            rcp_all = glob.tile([16, NQB, QBW], BF16, tag="rcp_all")
            rcp_f32 = glob.tile([16, QBW], F32, tag="rcp_f32")
            rcp_scr = glob.tile([16, QBW], F32, tag="rcp_scr")
            qs_all = [
                glob.tile([128, S], BF16, tag="qs", name=f"qs{i}", bufs=HQ)
                for i in range(HQ)
            ]

            nc.vector.memset(v_s[:, :, 64], 1.0)
            nc.vector.memset(sums_q[:], 1.0)

            with (
                tc.tile_pool(name="p1", bufs=1) as p1,
                tc.tile_pool(name="xp", bufs=3) as xp,
                tc.tile_pool(name="tmpp", bufs=4) as tmpp,
                tc.tile_pool(name="ptp", bufs=16) as ptp,
                tc.tile_pool(name="stgp", bufs=2) as stgp,
                tc.tile_pool(name="yp", bufs=4) as yp,
                tc.tile_pool(name="pssc", bufs=2, space="PSUM") as pssc,
                tc.tile_pool(name="psop", bufs=1, space="PSUM") as psop,
                tc.tile_pool(name="psml", bufs=3, space="PSUM") as psml,
            ):
                wq_s = p1.tile([128, DCH, 384], BF16, tag="wq_s")
                wqkv_r = wqkv_d.rearrange("(ko p) n -> p ko n", p=128)
                cos_s = p1.tile([128, S], BF16, tag="cos_s")
                sin_s = p1.tile([128, S], BF16, tag="sin_s")
                kvraw = p1.tile([128, S], F32, tag="kvraw")
                kswap = p1.tile([64, S], F32, tag="kswap")
                xt_r = xt_d.rearrange("(ko p) s -> p ko s", p=128)

                xblks = {}

                def issue_xblk(sb):
                    sbc = slice(sb * SBW, (sb + 1) * SBW)
                    xblk = xp.tile([128, DCH, SBW], BF16, tag="xblk", name=f"xblk{sb}")
                    for kq in range(4):
                        eng = nc.sync if kq % 2 == 0 else nc.gpsimd
                        eng.dma_start(
                            xblk[:, 4 * kq : 4 * kq + 4, :],
                            xt_r[:, 4 * kq : 4 * kq + 4, sbc],
                        )
                    xblks[sb] = xblk

                # ---------- projection for s-block sb, split into filler steps ----
                def proj_closures(sb):
                    sbc = slice(sb * SBW, (sb + 1) * SBW)
                    st = {}

                    def c_kv():
                        psKV = psml.tile([128, SBW], F32, tag="ps", name=f"psKV{sb}")
                        for kc in range(DCH):
                            nc.tensor.matmul(
                                psKV[:],
                                lhsT=wq_s[:, kc, 256:384],
                                rhs=xblks[sb][:, kc, :],
                                start=(kc == 0),
                                stop=(kc == DCH - 1),
                            )
                        nc.scalar.activation(kvraw[:, sbc], psKV[:], AF.Copy)

                    def c_a():
                        st["psA"] = psml.tile([128, SBW], F32, tag="ps", name=f"psA{sb}")
                        for kc in range(DCH):
                            nc.tensor.matmul(
                                st["psA"][:],
                                lhsT=wq_s[:, kc, 0:128],
                                rhs=xblks[sb][:, kc, :],
                                start=(kc == 0),
                                stop=(kc == DCH - 1),
                            )

                    def c_b():
                        st["psB"] = psml.tile([128, SBW], F32, tag="ps", name=f"psB{sb}")
                        for kc in range(DCH):
                            nc.tensor.matmul(
                                st["psB"][:],
                                lhsT=wq_s[:, kc, 128:256],
                                rhs=xblks[sb][:, kc, :],
                                start=(kc == 0),
                                stop=(kc == DCH - 1),
                            )

                    def c_ropeA():
                        psA, psB = st["psA"], st["psB"]
                        tmp = tmpp.tile([128, SBW], F32, tag="tmp", name=f"tmpA{sb}")
                        nc.vector.tensor_tensor(
                            outA[:, sbc], psA[:], cos_s[:, sbc], ALU.mult
                        )
                        nc.vector.tensor_tensor(tmp[:], psB[:], sin_s[:, sbc], ALU.mult)
                        nc.vector.tensor_tensor(
                            outA[:, sbc], outA[:, sbc], tmp[:], ALU.subtract
                        )

                    def c_ropeB():
                        psA, psB = st["psA"], st["psB"]
                        tmp2 = tmpp.tile([128, SBW], F32, tag="tmp", name=f"tmpB{sb}")
                        nc.vector.tensor_tensor(
                            outB[:, sbc], psB[:], cos_s[:, sbc], ALU.mult
                        )
                        nc.vector.tensor_tensor(
                            tmp2[:], psA[:], sin_s[:, sbc], ALU.mult
                        )
                        nc.vector.tensor_tensor(
                            outB[:, sbc], outB[:, sbc], tmp2[:], ALU.add
                        )

                    def c_krope():
                        nc.sync.dma_start(kswap[0:32, sbc], kvraw[32:64, sbc])
                        nc.sync.dma_start(kswap[32:64, sbc], kvraw[0:32, sbc])
                        nc.vector.tensor_tensor(
                            ktdup[0:64, sbc],
                            kvraw[0:64, sbc],
                            cos_s[0:64, sbc],
                            ALU.mult,
                        )
                        tmpk = tmpp.tile([64, SBW], F32, tag="tmpk", name=f"tmpk{sb}")
                        nc.vector.tensor_tensor(
                            tmpk[:], kswap[:, sbc], sin_s[0:64, sbc], ALU.mult
                        )
                        nc.vector.tensor_tensor(
                            ktdup[0:32, sbc],
                            ktdup[0:32, sbc],
                            tmpk[0:32, :],
                            ALU.subtract,
                        )
                        nc.vector.tensor_tensor(
                            ktdup[32:64, sbc],
                            ktdup[32:64, sbc],
                            tmpk[32:64, :],
                            ALU.add,
                        )
                        nc.sync.dma_start(ktdup[64:128, sbc], ktdup[0:64, sbc])

                    def c_vtrans():
                        for c in range(4 * sb, 4 * sb + 4):
                            ptr = psml.tile([128, SBW], F32, tag="ps", name=f"ptr{sb}_{c}")
                            nc.tensor.transpose(
                                ptr[:, 0:64],
                                kvraw[64:128, c * 128 : (c + 1) * 128],
                                ident[64:128, 64:128],
                            )
                            nc.vector.tensor_copy(v_s[:, c, 0:64], ptr[:, 0:64])

                    def c_qstage():
                        for h in range(HQ):
                            hc = slice(32 * h, 32 * h + 32)
                            qs = qs_all[h]
                            eng = nc.sync if h % 2 == 0 else nc.gpsimd
                            eng.dma_start(qs[0:32, sbc], outA[hc, sbc])
                            eng.dma_start(qs[32:64, sbc], outB[hc, sbc])
                            eng.dma_start(qs[64:96, sbc], outA[hc, sbc])
                            eng.dma_start(qs[96:128, sbc], outB[hc, sbc])

                    return [c_kv, c_a, c_b, c_ropeA, c_ropeB, c_krope, c_vtrans, c_qstage]

                # ---------------- attention unit: front (scores+exp+mask) --------
                def emit_front(h, qb):
                    qs = qs_all[h]
                    q0 = qb * QBW
                    nkc = 4 * (qb + 1)
                    pairs = []
                    for pair in range(nkc // 2):
                        cA, cB = 2 * pair, 2 * pair + 1
                        psc = pssc.tile([128, 1024], F32, tag="psc", name=f"psc{h}_{qb}_{pair}")
                        ptt = ptp.tile([128, 1024], BF16, tag="ptt", name=f"ptt{h}_{qb}_{pair}")
                        for c, half, r0 in ((cA, 0, 0), (cB, 1, 64)):
                            kc0 = c * 128
                            d = max(0, kc0 - q0)
                            nc.tensor.matmul(
                                psc[:, half * 512 + d : half * 512 + 512],
                                lhsT=ktdup[r0 : r0 + 64, kc0 : kc0 + 128],
                                rhs=qs[r0 : r0 + 64, q0 + d : q0 + QBW],
                                start=True,
                                stop=True,
                                tile_position=(r0, 0),
                            )
                        dA = max(0, cA * 128 - q0)
                        # single exp over [dA:1024]; the invalid middle region
                        # becomes garbage exp values that affine_select zeroes.
                        nc.scalar.activation(ptt[:, dA:1024], psc[:, dA:1024], AF.Exp)
                        for c, half in ((cA, 0), (cB, 1)):
                            kc0 = c * 128
                            if kc0 + 127 > q0:
                                ww = min(512, (kc0 - q0) + 128)
                                sl = slice(half * 512, half * 512 + ww)
                                nc.gpsimd.affine_select(
                                    out=ptt[:, sl],
                                    in_=ptt[:, sl],
                                    compare_op=ALU.is_ge,
                                    fill=0.0,
                                    base=q0 - kc0,
                                    channel_multiplier=-1,
                                    pattern=[[1, ww]],
                                )
                        pairs.append(ptt)
                    return pairs

                # ---------------- attention unit: back (PV + eviction) -----------
                def emit_back(h, qb, pairs):
                    nkc = 4 * (qb + 1)
                    pso = psop.tile([128, QBW], F32, tag="pso", name=f"pso{h}_{qb}")
                    for pair, ptt in enumerate(pairs):
                        for c, half in ((2 * pair, 0), (2 * pair + 1, 1)):
                            nc.tensor.matmul(
                                pso[0:65, :],
                                lhsT=v_s[:, c, :],
                                rhs=ptt[:, half * 512 : half * 512 + 512],
                                start=(c == 0),
                                stop=(c == nkc - 1),
                            )
                    ch = h // 2
                    rr = 64 * (h % 2)
                    stg = stgp.tile([128, QBW], BF16, tag="stg", name=f"stg{h}_{qb}")
                    nc.vector.tensor_copy(stg[0:64, :], pso[0:64, :])
                    sumr = stgp.tile([128, QBW], F32, tag="sumr", name=f"sumr{h}_{qb}")
                    nc.vector.tensor_copy(sumr[64:65, :], pso[64:65, :])
                    nc.sync.dma_start(
                        ao[rr : rr + 64, ch, qb * QBW : (qb + 1) * QBW], stg[0:64, :]
                    )
                    nc.sync.dma_start(sums_q[h : h + 1, qb, :], sumr[64:65, :])

                def emit_rcp(qb):
                    nc.vector.reciprocal_approx_accurate(
                        rcp_f32[:], sums_q[:, qb, :], rcp_scr[:]
                    )
                    nc.vector.tensor_copy(rcp_all[:, qb, :], rcp_f32[:])

                def norm_closure(h, qb):
                    def c():
                        ch = h // 2
                        rr = 64 * (h % 2)
                        q0 = qb * QBW
                        pbc = psml.tile([128, QBW], F32, tag="ps", name=f"pbc{h}_{qb}")
                        nc.tensor.matmul(
                            pbc[:],
                            lhsT=sel_s[:, h * 128 : (h + 1) * 128],
                            rhs=rcp_all[:, qb, :],
                            start=True,
                            stop=True,
                        )
                        nc.vector.tensor_tensor(
                            ao[rr : rr + 64, ch, q0 : q0 + QBW],
                            ao[rr : rr + 64, ch, q0 : q0 + QBW],
                            pbc[rr : rr + 64, :],
                            ALU.mult,
                        )

                    return c

                def oproj_closure(qb, i, act_every=3):
                    def c():
                        st = qb * 4 + i // 4
                        ob = i % 4
                        psy = psml.tile([128, QBW], F32, tag="ps", name=f"psy{qb}_{i}")
                        for chh in range(2):
                            nc.tensor.matmul(
                                psy[:],
                                lhsT=ao[:, chh, st * 128 : (st + 1) * 128],
                                rhs=wo_s[:, chh, ob * 512 : (ob + 1) * 512],
                                start=(chh == 0),
                                stop=(chh == 1),
                            )
                        ysb = yp.tile([128, QBW], BF16, tag="ysb", name=f"ysb{qb}_{i}")
                        if i % act_every == 0:
                            nc.scalar.activation(ysb[:], psy[:], AF.Copy)
                        else:
                            nc.vector.tensor_copy(ysb[:], psy[:])
                        eng = nc.gpsimd if i % 2 == 0 else nc.sync
                        eng.dma_start(
                            y_d[st * 128 : (st + 1) * 128, ob * 512 : (ob + 1) * 512],
                            ysb[:],
                        )

                    return c

                # ======================= prologue ================================
                for kc in range(DCH):
                    eng = nc.sync if kc % 2 == 0 else nc.gpsimd
                    eng.dma_start(wq_s[:, kc, :], wqkv_r[:, kc, :])
                issue_xblk(0)
                nc.sync.dma_start(cos_s[:], cos_d[:])
                nc.sync.dma_start(sin_s[:], sin_d[:])
                nc.gpsimd.dma_start(sel_s[:], sel_d[:])
                for chh in range(2):
                    nc.gpsimd.dma_start(wo_s[:, chh, :], wo_d[chh])
                make_identity(nc, ident[:])
                issue_xblk(1)
                for c in proj_closures(0):
                    c()
                issue_xblk(2)

                # ======================= pipelined stages ========================
                pending = [None]

                def flush_pending():
                    if pending[0] is not None:
                        emit_back(*pending[0])
                        pending[0] = None

                for qb in range(NQB):
                    fillers = []
                    if qb == 0:
                        fillers.append(lambda: issue_xblk(3))
                        fillers.extend(proj_closures(1))
                    elif qb == 1:
                        fillers.extend(proj_closures(2))
                        fillers.extend(norm_closure(h2, 0) for h2 in range(HQ))
                    elif qb == 2:
                        fillers.extend(proj_closures(3))
                        fillers.extend(norm_closure(h2, 1) for h2 in range(HQ))
                        fillers.extend(oproj_closure(0, i) for i in range(16))
                    else:
                        fillers.extend(oproj_closure(1, i) for i in range(8))
                        fillers.extend(norm_closure(h2, 2) for h2 in range(HQ))
                        fillers.extend(oproj_closure(1, i) for i in range(8, 16))
                        fillers.extend(oproj_closure(2, i) for i in range(8))

                    for h in range(HQ):
                        pairs = emit_front(h, qb)
                        flush_pending()
                        pending[0] = (h, qb, pairs)
                        if h == 0 and qb > 0:
                            emit_rcp(qb - 1)
                        npop = math.ceil(len(fillers) / (HQ - h))
                        for c in fillers[:npop]:
                            c()
                        fillers = fillers[npop:]
                    for c in fillers:
                        c()

                # ======================= epilogue ================================
                flush_pending()
                emit_rcp(3)
                for h in range(HQ):
                    norm_closure(h, 3)()
                for i in range(8, 16):
                    oproj_closure(2, i, act_every=2)()
                for i in range(16):
                    oproj_closure(3, i, act_every=2)()
    nc.compile()
    return nc


def _prep_inputs(x, Wq, Wk, Wv, Wo, inv_freq):
    """Host-side sharding + layout prep. Returns in_maps for the 8 cores."""
    x = np.ascontiguousarray(np.asarray(x, dtype=np.float32).reshape(S, D))
    xt = np.ascontiguousarray(x.T)  # [D, S]

    pos = np.arange(S, dtype=np.float64)
    inv = np.asarray(inv_freq, dtype=np.float64)  # [32]
    freqs = pos[None, :] * inv[:, None]  # [32, S]
    cos32 = np.cos(freqs).astype(np.float32)
    sin32 = np.sin(freqs).astype(np.float32)
    cos_tab = np.tile(cos32, (4, 1))  # [128, S]
    sin_tab = np.tile(sin32, (4, 1))
    sel = np.zeros((16, HQ * 128), dtype=np.float32)
    for h in range(HQ):
        rr = 64 * (h % 2)
        sel[h, h * 128 + rr : h * 128 + rr + 64] = 1.0

    in_maps = []
    for i in range(NCORES):
        wq_l = Wq[256 * i : 256 * (i + 1)].astype(np.float32) * 0.125  # [256, D]
        wk_l = Wk[64 * i : 64 * (i + 1)].astype(np.float32)  # [64, D]
        wv_l = Wv[64 * i : 64 * (i + 1)].astype(np.float32)  # [64, D]
        # A-tile: first-half dims of the 4 heads; B-tile: second halves
        wA = np.concatenate(
            [wq_l[64 * h : 64 * h + 32] for h in range(HQ)], axis=0
        )  # [128, D]
        wB = np.concatenate(
            [wq_l[64 * h + 32 : 64 * h + 64] for h in range(HQ)], axis=0
        )
        wkv = np.concatenate([wk_l, wv_l], axis=0)  # [128, D]
        wqkv = np.ascontiguousarray(
            np.concatenate([wA, wB, wkv], axis=0).T
        )  # [D, 384]
        wo_l = Wo[:, 256 * i : 256 * (i + 1)].astype(np.float32)  # [D, 256]
        wo_t = np.ascontiguousarray(wo_l.T.reshape(2, 128, D))  # [2, 128, D]
        in_maps.append(
            {
                "xt": xt.astype(ml_dtypes.bfloat16),
                "wqkv": wqkv.astype(ml_dtypes.bfloat16),
                "wo": wo_t.astype(ml_dtypes.bfloat16),
                "cos": cos_tab.astype(ml_dtypes.bfloat16),
                "sin": sin_tab.astype(ml_dtypes.bfloat16),
                "sel": sel.astype(ml_dtypes.bfloat16),
            }
        )
    return in_maps


_NC_CACHE = None


def kernel(x, Wq, Wk, Wv, Wo, inv_freq):
    global _NC_CACHE
    if _NC_CACHE is None:
        _NC_CACHE = _build_nc()
    nc = _NC_CACHE
    in_maps = _prep_inputs(x, Wq, Wk, Wv, Wo, inv_freq)
    trace = bool(int(os.environ.get("BASS_KERNEL_TRACE", "0")))
    res = None
    last_exc = None
    for attempt in range(3):
        try:
            res = run_bass_kernel_spmd(nc, in_maps, list(range(NCORES)), trace=trace)
            break
        except Exception as e:  # transient device faults (rare) — retry
            last_exc = e
            msg = str(e)
            if "UNRECOVERABLE" in msg or "UNAVAILABLE" in msg or "Timeout" in msg:
                continue
            raise
    if res is None:
        raise last_exc
    if trace:
        kernel.last_results = res
    y = np.zeros((S, D), dtype=np.float32)
    for i in range(NCORES):
        y += res.results[i]["y"].astype(np.float32)
    return y.reshape(1, S, D)


# revision 16
# speedup vs baseline: 1.1567x; 1.0416x over previous
"""Trainium2 Bass kernel for GQA attention (B=1, S=2048, D=2048, H=32, KV=8, HD=64).

Tensor-parallel over heads across 8 NeuronCores: core i holds q-heads
[4i, 4i+4) and kv-head i; each core computes its partial o_proj output and the
host sums the 8 partials (Megatron all-reduce done host-side).

Pipelined emission: q-block-major stages; each attention unit's scores/exp run
one unit ahead of its PV matmuls, with projection and o_proj matmuls
interleaved as PE filler so the tensor engine never idles (keeps HAM warm).

Self-contained: only imports concourse (on sys.path in the container).
"""

import math
import os
import sys

import ml_dtypes
import numpy as np

if "/opt/trn_rl_repo" not in sys.path and not any(
    p.endswith("trn_rl_repo") for p in sys.path
):
    sys.path.insert(0, "/opt/trn_rl_repo")

import concourse.bass as bass
import concourse.mybir as mybir
import concourse.tile as tile
from concourse import bacc
from concourse.bass_utils import run_bass_kernel_spmd
from concourse.masks import make_identity

F32 = mybir.dt.float32
BF16 = mybir.dt.bfloat16

AF = mybir.ActivationFunctionType
ALU = mybir.AluOpType

S = 2048
D = 2048
H = 32
KV = 8
HD = 64
NCORES = 8
HQ = H // NCORES  # 4 q heads per core
NQB = 4  # q blocks of 512
QBW = 512
NSB = 4  # s blocks of 512 in projection
SBW = 512
DCH = D // 128  # 16 contraction chunks


def _build_nc():
    nc = bacc.Bacc("TRN2", target_bir_lowering=False, debug=False, num_devices=NCORES)

    xt_d = nc.declare_dram_parameter("xt", [D, S], BF16, isOutput=False)
    wqkv_d = nc.declare_dram_parameter("wqkv", [D, 384], BF16, isOutput=False)
    wo_d = nc.declare_dram_parameter("wo", [2, 128, D], BF16, isOutput=False)
    cos_d = nc.declare_dram_parameter("cos", [128, S], BF16, isOutput=False)
    sin_d = nc.declare_dram_parameter("sin", [128, S], BF16, isOutput=False)
    sel_d = nc.declare_dram_parameter("sel", [16, HQ * 128], BF16, isOutput=False)
    y_d = nc.declare_dram_parameter("y", [S, D], BF16, isOutput=True)

    with tile.TileContext(nc) as tc:
        with (
            tc.tile_pool(name="glob", bufs=1) as glob,
        ):
            ktdup = glob.tile([128, S], BF16, tag="ktdup")
            v_s = glob.tile([128, 16, 65], BF16, tag="v_s")
            outA = glob.tile([128, S], BF16, tag="outA")
            outB = glob.tile([128, S], BF16, tag="outB")
            ao = glob.tile([128, 2, S], BF16, tag="ao")
            sel_s = glob.tile([16, HQ * 128], BF16, tag="sel_s")
            ident = glob.tile([128, 128], F32, tag="ident")
            wo_s = glob.tile([128, 2, D], BF16, tag="wo_s")
            sums_q = glob.tile([16, NQB, QBW], F32, tag="sums_q")
            rcp_all = glob.tile([16, NQB, QBW], BF16, tag="rcp_all")
            rcp_f32 = glob.tile([16, QBW], F32, tag="rcp_f32")
            rcp_scr = glob.tile([16, QBW], F32, tag="rcp_scr")
            qs_all = [
                glob.tile([128, S], BF16, tag="qs", name=f"qs{i}", bufs=HQ)
                for i in range(HQ)
            ]

            nc.vector.memset(v_s[:, :, 64], 1.0)
            nc.vector.memset(sums_q[:], 1.0)

            with (
                tc.tile_pool(name="p1", bufs=1) as p1,
                tc.tile_pool(name="xp", bufs=3) as xp,
                tc.tile_pool(name="tmpp", bufs=4) as tmpp,
                tc.tile_pool(name="ptp", bufs=16) as ptp,
                tc.tile_pool(name="stgp", bufs=2) as stgp,
                tc.tile_pool(name="yp", bufs=4) as yp,
                tc.tile_pool(name="pssc", bufs=2, space="PSUM") as pssc,
                tc.tile_pool(name="psop", bufs=1, space="PSUM") as psop,
                tc.tile_pool(name="psml", bufs=3, space="PSUM") as psml,
            ):
                wq_s = p1.tile([128, DCH, 384], BF16, tag="wq_s")
                wqkv_r = wqkv_d.rearrange("(ko p) n -> p ko n", p=128)
                cos_s = p1.tile([128, S], BF16, tag="cos_s")
                sin_s = p1.tile([128, S], BF16, tag="sin_s")
                kvraw = p1.tile([128, S], F32, tag="kvraw")
                kswap = p1.tile([64, S], F32, tag="kswap")
                xt_r = xt_d.rearrange("(ko p) s -> p ko s", p=128)

                xblks = {}

                def issue_xblk(sb):
                    sbc = slice(sb * SBW, (sb + 1) * SBW)
                    xblk = xp.tile([128, DCH, SBW], BF16, tag="xblk", name=f"xblk{sb}")
                    if sb == 0:
                        # split for pipelining: first MMs start when group 0 lands
                        for kq in range(4):
                            eng = nc.sync if kq % 2 == 0 else nc.gpsimd
                            eng.dma_start(
                                xblk[:, 4 * kq : 4 * kq + 4, :],
                                xt_r[:, 4 * kq : 4 * kq + 4, sbc],
                            )
                    else:
                        nc.sync.dma_start(xblk[:], xt_r[:, :, sbc])
                    xblks[sb] = xblk

                # ---------- projection for s-block sb, split into filler steps ----
                def proj_closures(sb):
                    sbc = slice(sb * SBW, (sb + 1) * SBW)
                    st = {}

                    def c_kv():
                        psKV = psml.tile([128, SBW], F32, tag="ps", name=f"psKV{sb}")
                        for kc in range(DCH):
                            nc.tensor.matmul(
                                psKV[:],
                                lhsT=wq_s[:, kc, 256:384],
                                rhs=xblks[sb][:, kc, :],
                                start=(kc == 0),
                                stop=(kc == DCH - 1),
                            )
                        nc.scalar.activation(kvraw[:, sbc], psKV[:], AF.Copy)

                    def c_a():
                        st["psA"] = psml.tile([128, SBW], F32, tag="ps", name=f"psA{sb}")
                        for kc in range(DCH):
                            nc.tensor.matmul(
                                st["psA"][:],
                                lhsT=wq_s[:, kc, 0:128],
                                rhs=xblks[sb][:, kc, :],
                                start=(kc == 0),
                                stop=(kc == DCH - 1),
                            )

                    def c_b():
                        st["psB"] = psml.tile([128, SBW], F32, tag="ps", name=f"psB{sb}")
                        for kc in range(DCH):
                            nc.tensor.matmul(
                                st["psB"][:],
                                lhsT=wq_s[:, kc, 128:256],
                                rhs=xblks[sb][:, kc, :],
                                start=(kc == 0),
                                stop=(kc == DCH - 1),
                            )

                    def c_ropeA():
                        psA, psB = st["psA"], st["psB"]
                        tmp = tmpp.tile([128, SBW], F32, tag="tmp", name=f"tmpA{sb}")
                        nc.vector.tensor_tensor(
                            outA[:, sbc], psA[:], cos_s[:, sbc], ALU.mult
                        )
                        nc.vector.tensor_tensor(tmp[:], psB[:], sin_s[:, sbc], ALU.mult)
                        nc.vector.tensor_tensor(
                            outA[:, sbc], outA[:, sbc], tmp[:], ALU.subtract
                        )

                    def c_ropeB():
                        psA, psB = st["psA"], st["psB"]
                        tmp2 = tmpp.tile([128, SBW], F32, tag="tmp", name=f"tmpB{sb}")
                        nc.vector.tensor_tensor(
                            outB[:, sbc], psB[:], cos_s[:, sbc], ALU.mult
                        )
                        nc.vector.tensor_tensor(
                            tmp2[:], psA[:], sin_s[:, sbc], ALU.mult
                        )
                        nc.vector.tensor_tensor(
                            outB[:, sbc], outB[:, sbc], tmp2[:], ALU.add
                        )

                    def c_krope():
                        nc.sync.dma_start(kswap[0:32, sbc], kvraw[32:64, sbc])
                        nc.sync.dma_start(kswap[32:64, sbc], kvraw[0:32, sbc])
                        nc.vector.tensor_tensor(
                            ktdup[0:64, sbc],
                            kvraw[0:64, sbc],
                            cos_s[0:64, sbc],
                            ALU.mult,
                        )
                        tmpk = tmpp.tile([64, SBW], F32, tag="tmpk", name=f"tmpk{sb}")
                        nc.vector.tensor_tensor(
                            tmpk[:], kswap[:, sbc], sin_s[0:64, sbc], ALU.mult
                        )
                        nc.vector.tensor_tensor(
                            ktdup[0:32, sbc],
                            ktdup[0:32, sbc],
                            tmpk[0:32, :],
                            ALU.subtract,
                        )
                        nc.vector.tensor_tensor(
                            ktdup[32:64, sbc],
                            ktdup[32:64, sbc],
                            tmpk[32:64, :],
                            ALU.add,
                        )
                        nc.sync.dma_start(ktdup[64:128, sbc], ktdup[0:64, sbc])

                    def c_vtrans():
                        for c in range(4 * sb, 4 * sb + 4):
                            ptr = psml.tile([128, SBW], F32, tag="ps", name=f"ptr{sb}_{c}")
                            nc.tensor.transpose(
                                ptr[:, 0:64],
                                kvraw[64:128, c * 128 : (c + 1) * 128],
                                ident[64:128, 64:128],
                            )
                            nc.vector.tensor_copy(v_s[:, c, 0:64], ptr[:, 0:64])

                    def c_qstage():
                        for h in range(HQ):
                            hc = slice(32 * h, 32 * h + 32)
                            qs = qs_all[h]
                            eng = nc.sync if h % 2 == 0 else nc.gpsimd
                            eng.dma_start(qs[0:32, sbc], outA[hc, sbc])
                            eng.dma_start(qs[32:64, sbc], outB[hc, sbc])
                            eng.dma_start(qs[64:96, sbc], outA[hc, sbc])
                            eng.dma_start(qs[96:128, sbc], outB[hc, sbc])

                    return [c_kv, c_a, c_b, c_ropeA, c_ropeB, c_krope, c_vtrans, c_qstage]

                # ---------------- attention unit: front (scores+exp+mask) --------
                def emit_front(h, qb):
                    qs = qs_all[h]
                    q0 = qb * QBW
                    nkc = 4 * (qb + 1)
                    pairs = []
                    for pair in range(nkc // 2):
                        cA, cB = 2 * pair, 2 * pair + 1
                        psc = pssc.tile([128, 1024], F32, tag="psc", name=f"psc{h}_{qb}_{pair}")
                        ptt = ptp.tile([128, 1024], BF16, tag="ptt", name=f"ptt{h}_{qb}_{pair}")
                        for c, half, r0 in ((cA, 0, 0), (cB, 1, 64)):
                            kc0 = c * 128
                            d = max(0, kc0 - q0)
                            nc.tensor.matmul(
                                psc[:, half * 512 + d : half * 512 + 512],
                                lhsT=ktdup[r0 : r0 + 64, kc0 : kc0 + 128],
                                rhs=qs[r0 : r0 + 64, q0 + d : q0 + QBW],
                                start=True,
                                stop=True,
                                tile_position=(r0, 0),
                            )
                        dA = max(0, cA * 128 - q0)
                        # single exp over [dA:1024]; the invalid middle region
                        # becomes garbage exp values that affine_select zeroes.
                        nc.scalar.activation(ptt[:, dA:1024], psc[:, dA:1024], AF.Exp)
                        for c, half in ((cA, 0), (cB, 1)):
                            kc0 = c * 128
                            if kc0 + 127 > q0:
                                ww = min(512, (kc0 - q0) + 128)
                                sl = slice(half * 512, half * 512 + ww)
                                nc.gpsimd.affine_select(
                                    out=ptt[:, sl],
                                    in_=ptt[:, sl],
                                    compare_op=ALU.is_ge,
                                    fill=0.0,
                                    base=q0 - kc0,
                                    channel_multiplier=-1,
                                    pattern=[[1, ww]],
                                )
                        pairs.append(ptt)
                    return pairs

                # ---------------- attention unit: back (PV + eviction) -----------
                def emit_back(h, qb, pairs):
                    nkc = 4 * (qb + 1)
                    pso = psop.tile([128, QBW], F32, tag="pso", name=f"pso{h}_{qb}")
                    for pair, ptt in enumerate(pairs):
                        for c, half in ((2 * pair, 0), (2 * pair + 1, 1)):
                            nc.tensor.matmul(
                                pso[0:65, :],
                                lhsT=v_s[:, c, :],
                                rhs=ptt[:, half * 512 : half * 512 + 512],
                                start=(c == 0),
                                stop=(c == nkc - 1),
                            )
                    ch = h // 2
                    rr = 64 * (h % 2)
                    stg = stgp.tile([128, QBW], BF16, tag="stg", name=f"stg{h}_{qb}")
                    nc.vector.tensor_copy(stg[0:64, :], pso[0:64, :])
                    sumr = stgp.tile([128, QBW], F32, tag="sumr", name=f"sumr{h}_{qb}")
                    nc.vector.tensor_copy(sumr[64:65, :], pso[64:65, :])
                    nc.sync.dma_start(
                        ao[rr : rr + 64, ch, qb * QBW : (qb + 1) * QBW], stg[0:64, :]
                    )
                    nc.sync.dma_start(sums_q[h : h + 1, qb, :], sumr[64:65, :])

                def emit_rcp(qb):
                    nc.vector.reciprocal_approx_accurate(
                        rcp_f32[:], sums_q[:, qb, :], rcp_scr[:]
                    )
                    nc.vector.tensor_copy(rcp_all[:, qb, :], rcp_f32[:])

                def norm_closure(h, qb):
                    def c():
                        ch = h // 2
                        rr = 64 * (h % 2)
                        q0 = qb * QBW
                        pbc = psml.tile([128, QBW], F32, tag="ps", name=f"pbc{h}_{qb}")
                        nc.tensor.matmul(
                            pbc[:],
                            lhsT=sel_s[:, h * 128 : (h + 1) * 128],
                            rhs=rcp_all[:, qb, :],
                            start=True,
                            stop=True,
                        )
                        nc.vector.tensor_tensor(
                            ao[rr : rr + 64, ch, q0 : q0 + QBW],
                            ao[rr : rr + 64, ch, q0 : q0 + QBW],
                            pbc[rr : rr + 64, :],
                            ALU.mult,
                        )

                    return c

                def oproj_closure(qb, i, act_every=3):
                    def c():
                        st = qb * 4 + i // 4
                        ob = i % 4
                        psy = psml.tile([128, QBW], F32, tag="ps", name=f"psy{qb}_{i}")
                        for chh in range(2):
                            nc.tensor.matmul(
                                psy[:],
                                lhsT=ao[:, chh, st * 128 : (st + 1) * 128],
                                rhs=wo_s[:, chh, ob * 512 : (ob + 1) * 512],
                                start=(chh == 0),
                                stop=(chh == 1),
                            )
                        ysb = yp.tile([128, QBW], BF16, tag="ysb", name=f"ysb{qb}_{i}")
                        if i % act_every == 0:
                            nc.scalar.activation(ysb[:], psy[:], AF.Copy)
                        else:
                            nc.vector.tensor_copy(ysb[:], psy[:])
                        eng = nc.gpsimd if i % 2 == 0 else nc.sync
                        eng.dma_start(
                            y_d[st * 128 : (st + 1) * 128, ob * 512 : (ob + 1) * 512],
                            ysb[:],
                        )

                    return c

                # ======================= prologue ================================
                nc.sync.dma_start(wq_s[:, 0:4, :], wqkv_r[:, 0:4, :])
                issue_xblk(0)
                for kg in range(1, 4):
                    eng = nc.gpsimd if kg % 2 == 0 else nc.sync
                    eng.dma_start(
                        wq_s[:, 4 * kg : 4 * kg + 4, :],
                        wqkv_r[:, 4 * kg : 4 * kg + 4, :],
                    )
                nc.sync.dma_start(cos_s[:], cos_d[:])
                nc.sync.dma_start(sin_s[:], sin_d[:])
                nc.gpsimd.dma_start(sel_s[:], sel_d[:])
                for chh in range(2):
                    nc.gpsimd.dma_start(wo_s[:, chh, :], wo_d[chh])
                make_identity(nc, ident[:])
                issue_xblk(1)
                for c in proj_closures(0):
                    c()
                issue_xblk(2)

                # ======================= pipelined stages ========================
                pending = [None]

                def flush_pending():
                    if pending[0] is not None:
                        emit_back(*pending[0])
                        pending[0] = None

                for qb in range(NQB):
                    fillers = []
                    if qb == 0:
                        fillers.append(lambda: issue_xblk(3))
                        fillers.extend(proj_closures(1))
                    elif qb == 1:
                        fillers.extend(proj_closures(2))
                        fillers.extend(norm_closure(h2, 0) for h2 in range(HQ))
                    elif qb == 2:
                        fillers.extend(proj_closures(3))
                        fillers.extend(norm_closure(h2, 1) for h2 in range(HQ))
                        fillers.extend(oproj_closure(0, i) for i in range(16))
                    else:
                        fillers.extend(oproj_closure(1, i) for i in range(8))
                        fillers.extend(norm_closure(h2, 2) for h2 in range(HQ))
                        fillers.extend(oproj_closure(1, i) for i in range(8, 16))
                        fillers.extend(oproj_closure(2, i) for i in range(8))

                    for h in range(HQ):
                        pairs = emit_front(h, qb)
                        flush_pending()
                        pending[0] = (h, qb, pairs)
                        if h == 0 and qb > 0:
                            emit_rcp(qb - 1)
                        npop = math.ceil(len(fillers) / (HQ - h))
                        for c in fillers[:npop]:
                            c()
                        fillers = fillers[npop:]
                    for c in fillers:
                        c()

                # ======================= epilogue ================================
                flush_pending()
                emit_rcp(3)
                for h in range(HQ):
                    norm_closure(h, 3)()
                for i in range(8, 16):
                    oproj_closure(2, i, act_every=2)()
                for i in range(16):
                    oproj_closure(3, i, act_every=2)()
    nc.compile()
    return nc


def _prep_inputs(x, Wq, Wk, Wv, Wo, inv_freq):
    """Host-side sharding + layout prep. Returns in_maps for the 8 cores."""
    x = np.ascontiguousarray(np.asarray(x, dtype=np.float32).reshape(S, D))
    xt = np.ascontiguousarray(x.T)  # [D, S]

    pos = np.arange(S, dtype=np.float64)
    inv = np.asarray(inv_freq, dtype=np.float64)  # [32]
    freqs = pos[None, :] * inv[:, None]  # [32, S]
    cos32 = np.cos(freqs).astype(np.float32)
    sin32 = np.sin(freqs).astype(np.float32)
    cos_tab = np.tile(cos32, (4, 1))  # [128, S]
    sin_tab = np.tile(sin32, (4, 1))
    sel = np.zeros((16, HQ * 128), dtype=np.float32)
    for h in range(HQ):
        rr = 64 * (h % 2)
        sel[h, h * 128 + rr : h * 128 + rr + 64] = 1.0

    in_maps = []
    for i in range(NCORES):
        wq_l = Wq[256 * i : 256 * (i + 1)].astype(np.float32) * 0.125  # [256, D]
        wk_l = Wk[64 * i : 64 * (i + 1)].astype(np.float32)  # [64, D]
        wv_l = Wv[64 * i : 64 * (i + 1)].astype(np.float32)  # [64, D]
        # A-tile: first-half dims of the 4 heads; B-tile: second halves
        wA = np.concatenate(
            [wq_l[64 * h : 64 * h + 32] for h in range(HQ)], axis=0
        )  # [128, D]
        wB = np.concatenate(
            [wq_l[64 * h + 32 : 64 * h + 64] for h in range(HQ)], axis=0
        )
        wkv = np.concatenate([wk_l, wv_l], axis=0)  # [128, D]
        wqkv = np.ascontiguousarray(
            np.concatenate([wA, wB, wkv], axis=0).T
        )  # [D, 384]
        wo_l = Wo[:, 256 * i : 256 * (i + 1)].astype(np.float32)  # [D, 256]
        wo_t = np.ascontiguousarray(wo_l.T.reshape(2, 128, D))  # [2, 128, D]
        in_maps.append(
            {
                "xt": xt.astype(ml_dtypes.bfloat16),
                "wqkv": wqkv.astype(ml_dtypes.bfloat16),
                "wo": wo_t.astype(ml_dtypes.bfloat16),
                "cos": cos_tab.astype(ml_dtypes.bfloat16),
                "sin": sin_tab.astype(ml_dtypes.bfloat16),
                "sel": sel.astype(ml_dtypes.bfloat16),
            }
        )
    return in_maps


_NC_CACHE = None


def kernel(x, Wq, Wk, Wv, Wo, inv_freq):
    global _NC_CACHE
    if _NC_CACHE is None:
        _NC_CACHE = _build_nc()
    nc = _NC_CACHE
    in_maps = _prep_inputs(x, Wq, Wk, Wv, Wo, inv_freq)
    trace = bool(int(os.environ.get("BASS_KERNEL_TRACE", "0")))
    res = None
    last_exc = None
    for attempt in range(3):
        try:
            res = run_bass_kernel_spmd(nc, in_maps, list(range(NCORES)), trace=trace)
            break
        except Exception as e:  # transient device faults (rare) — retry
            last_exc = e
            msg = str(e)
            if "UNRECOVERABLE" in msg or "UNAVAILABLE" in msg or "Timeout" in msg:
                continue
            raise
    if res is None:
        raise last_exc
    if trace:
        kernel.last_results = res
    y = np.zeros((S, D), dtype=np.float32)
    for i in range(NCORES):
        y += res.results[i]["y"].astype(np.float32)
    return y.reshape(1, S, D)
